# revision 24
# baseline (speedup 1.0000x reference)
"""BinaryMatchAttention Trainium2 kernel.

reference semantics (per batch b):
    qb[k]   = (query_addr >> k) & 1                 k in [0, 16)
    w[s]    = prod_k (1 - |x[b, s, 96+k] - qb[k]|)
    out[b,d]= sum_s w[s] * x[b, s, d]               d in [0, 96)

Sharding: data-parallel over batch, one NeuronCore per batch element
(B == 8 == n_cores), no collectives.

Per-core plan (x_core [32768, 128] fp32 in HBM):
  - slab layout: partition p holds the 256 consecutive rows
    s = p*256 + i.  Each DMA wave loads a block of rows [r0, r0+Wr) for
    all partitions; the per-partition source is one contiguous Wr*512 B
    run in HBM, so every wave is 128 large contiguous descriptors
    (up to 24 KiB each) instead of many 2 KiB ones.  Per-SDMA-engine
    rate goes from ~24 GB/s (2 KiB descs) to ~26 GB/s, near the ~27
    GB/s port limit (~410 GB/s aggregate when the sibling NeuronCore
    on the same HBM stack is not streaming concurrently).
  - all data waves ride ONE HWDGE ring (see WS_ROWS comment): strict
    FIFO makes waves complete in program order for the in-order DVE
    consumer, and the wave schedule ramps 8 -> 48 rows at the head
    (compute starts ~2 us after the first bytes land) and tapers
    48 -> 4 at the tail so each wave's weight chain finishes before
    the next wave's completion lands; the post-stream drain is just
    the 4-row chain + one matmul + PSUM drain.
  - match weights, per wave: d = bits - qb (DVE, fp16 out),
    na = min(-d, d) = -|d| (DVE stt), m = 1 + na (DVE), then ONE
    product-reduction over the 16 factors (DVE tensor_reduce mult)
    giving w[p, i] in f32r.  fp16 intermediates add ~1.5e-3 L2 error,
    well inside the 2e-2 gate, and cannot overflow (sum log2|m_k| is
    far below 16 for N(0,1) inputs).
  - einsum on TensorE: per 4-row group,
    psum[4, 384] += w[:, 4j:4j+4].T @ v[:, 4j:4j+4, 0:96]  (float32r,
    1 cycle/row at N=384), accumulated across all 64 groups in one
    PSUM bank.  Only the diagonal 96-blocks (r == r') are wanted; the
    host extracts and sums them (24 junk floats per row ignored).
  - mode "f32" is an exact-fp32 fallback (fp32 weight chain, fp32 PE).
"""

import os
import sys

if "/opt/trn_rl_repo" not in sys.path:
    sys.path.insert(0, "/opt/trn_rl_repo")

import numpy as np

S, D = 32768, 128
VD = 96          # value payload dims
NBITS = 16
BIT0 = 96
P = 128          # partitions
IPP = S // P     # 256 seq rows per partition (slab layout)
R = 4            # seq rows per matmul (diagonal trick)
C = R

# Wave sizes in rows-per-partition.  All data waves ride ONE HWDGE
# ring (Sync's qSPDynamicHW): the SDMA engines drain a ring strictly
# FIFO, so waves complete in program order and the in-order DVE/PE
# consumers never stall on a late out-of-order wave (two rings
# interleave per-descriptor, which starves whichever ring holds the
# smaller descriptors).  Head ramps 8 -> 48 rows so compute starts as
# soon as the first bytes land; tail tapers 48 -> 4 so each wave's
# DVE chain (~rows*0.07 + 0.35 us) finishes before the next wave's
# completion lands (next_rows * 0.158 us later), leaving only a
# 4-row chain + one matmul + PSUM drain after the last DMA byte.
WS_ROWS = [8, 8, 16] + [48, 48, 48] + [32, 24, 12, 8, 4]
assert sum(WS_ROWS) == IPP
assert all(w % R == 0 for w in WS_ROWS)

NCORES = 8

# "f32r" : float32r matmuls (1 cycle/row, ~5e-4 rel err)
# "f32"  : plain fp32 matmuls (4 cycles/row, exact)
MM_MODE = os.environ.get("BMA_MM_MODE", "f32r")

_CACHE = {}


def _build(mode):
    import concourse.bacc as bacc
    import concourse.mybir as mybir
    import concourse.tile as tile

    f32 = mybir.dt.float32
    x_dt = mybir.dt.float32r if mode == "f32r" else f32
    # fp16 for the match-weight chain: 2x DVE throughput, and the
    # rounding error (~1e-3 L2 on the output) is far inside the 2e-2
    # gate.  No overflow risk: sum(log2|m_k|) stays far below 16.
    w_dt = mybir.dt.float16 if mode == "f32r" else f32

    nc = bacc.Bacc("TRN2", target_bir_lowering=False, debug=False)
    x = nc.dram_tensor("x", [S, D], x_dt, kind="ExternalInput")
    cq = nc.dram_tensor("cq", [P, NBITS], f32, kind="ExternalInput")
    out = nc.dram_tensor("out", [C, C * VD], f32, kind="ExternalOutput")

    # [128(part), 256*128]; partition p's source is one contiguous
    # 128 KiB run in HBM, so any row-block slice is a single large
    # contiguous descriptor per partition.
    xr = x.ap().rearrange("(p i) d -> p (i d)", p=P)

    n_mm = IPP // R  # 64 accumulating matmuls
    WMAX = max(WS_ROWS)
    NVT = 7  # DMA-ahead depth (manual ring of persistent vt tiles)
    with tile.TileContext(nc) as tc:
        with (
            tc.tile_pool(name="const", bufs=1) as cpool,
            tc.tile_pool(name="v", bufs=1) as vpool,
            tc.tile_pool(name="wk", bufs=1) as wpool,
            tc.tile_pool(name="ps", bufs=1, space="PSUM") as ppool,
            tc.tile_pool(name="res", bufs=1) as rpool,
        ):
            cqt = cpool.tile([P, 1, NBITS], f32)
            # cq rides the ACT HWDGE ring so wave 0's descriptors on
            # the SP ring are generated without queueing behind it.
            nc.scalar.dma_start(cqt[:], cq.ap().rearrange("p (a k) -> p a k", a=1))

            acc = ppool.tile([C, C * VD], f32)

            # Persistent tiles, sliced per wave, instead of per-wave
            # pool.tile() calls: every logical tile costs a release +
            # one semaphore hop PER ENGINE in Tile's end-of-kernel
            # drain (measured ~130 ns each x ~57 tiles = ~7.5 us of
            # teardown).  The chain tiles (d/na/m) are produced and
            # consumed only by the in-order DVE stream, so single
            # buffers add no stalls; w is read cross-engine by PE so
            # it double-buffers; vt keeps an NVT-deep manual ring for
            # DMA-ahead, equivalent to the old pool bufs.
            dt_ = wpool.tile([P, WMAX, NBITS], w_dt, tag="d")
            nat = wpool.tile([P, WMAX, NBITS], w_dt, tag="na")
            mt = wpool.tile([P, WMAX, NBITS], w_dt, tag="m")
            wts = [
                wpool.tile([P, WMAX, 1], x_dt, tag=f"w{i}", name=f"w{i}")
                for i in range(2)
            ]
            vts = [
                vpool.tile([P, WMAX * D], x_dt, tag=f"vt{i}", name=f"vt{i}")
                for i in range(NVT)
            ]

            g = 0
            r0 = 0
            for ib, Wr in enumerate(WS_ROWS):
                vt = vts[ib % NVT][:, 0 : Wr * D]
                nc.sync.dma_start(vt, xr[:, r0 * D : (r0 + Wr) * D])
                vr = vt.rearrange("p (i d) -> p i d", d=D)

                bits = vr[:, :, BIT0 : BIT0 + NBITS]
                if mode == "f32r":
                    bits = bits.bitcast(f32)
                d = dt_[:, 0:Wr, :]
                nc.vector.tensor_sub(d, bits, cqt[:].broadcast_to([P, Wr, NBITS]))
                # na = min(-d, d) = -|d| on DVE; ACT is kept free so its
                # HWDGE descriptor pushes are never blocked behind compute
                na = nat[:, 0:Wr, :]
                nc.vector.scalar_tensor_tensor(
                    na, d, -1.0, d,
                    op0=mybir.AluOpType.mult, op1=mybir.AluOpType.min,
                )
                m = mt[:, 0:Wr, :]
                nc.vector.tensor_scalar(
                    m, na, 1.0, None, op0=mybir.AluOpType.add,
                )
                # single DVE product-reduce over the 16 match factors,
                # written in the matmul dtype so the verifier sees an
                # f32r producer
                w = wts[ib % 2][:, 0:Wr, :]
                nc.vector.tensor_reduce(
                    w, m, axis=mybir.AxisListType.X,
                    op=mybir.AluOpType.mult,
                )

                for j in range(Wr // R):
                    lhsT = w[:, j * R : (j + 1) * R, 0]     # [128, 4]
                    rhs = vr[:, j * R : (j + 1) * R, 0:VD]  # [128, 4, 96]
                    nc.tensor.matmul(
                        acc[:],
                        lhsT,
                        rhs,
                        start=(g == 0),
                        stop=(g == n_mm - 1),
                    )
                    g += 1
                r0 += Wr

            res = rpool.tile([C, C * VD], f32)
            nc.vector.tensor_copy(res[:], acc[:])
            nc.sync.dma_start(out.ap(), res[:])

    nc.compile()
    return nc


def _get_nc(mode):
    if mode not in _CACHE:
        _CACHE[mode] = _build(mode)
    return _CACHE[mode]


def run(x, query_addr, trace=False, mode=None):
    """Returns (output [B, 96] float32, BassKernelResults)."""
    from concourse.bass_utils import run_bass_kernel_spmd

    mode = mode or MM_MODE
    x = np.asarray(x)
    qa = int(np.asarray(query_addr))
    assert x.shape == (NCORES, S, D), x.shape

    qb = np.array([(qa >> k) & 1 for k in range(NBITS)], dtype=np.float32)
    cq = np.ascontiguousarray(np.broadcast_to(qb, (P, NBITS)))

    nc = _get_nc(mode)
    in_maps = [
        {"x": np.ascontiguousarray(x[b], dtype=np.float32), "cq": cq}
        for b in range(NCORES)
    ]
    if not trace:
        # A stray BASS_TRACE in the env would route run_bass_kernel_spmd
        # into the NTFF-hook path, which needs antenv.axon_hooks (absent
        # in this image unless test.py installs a shim).
        os.environ["BASS_NEVER_TRACE"] = "1"
    else:
        os.environ.pop("BASS_NEVER_TRACE", None)
    kres = run_bass_kernel_spmd(nc, in_maps, list(range(NCORES)), trace=trace)

    outs = []
    for r in kres.results:
        o = np.asarray(r["out"]).reshape(C, C, VD)
        outs.append(o[np.arange(C), np.arange(C)].sum(axis=0))
    return np.stack(outs).astype(np.float32), kres


def kernel(x, query_addr):
    return run(x, query_addr)[0]


# revision 28
# speedup vs baseline: 1.1337x; 1.1337x over previous
"""BinaryMatchAttention Trainium2 kernel.

reference semantics (per batch b):
    qb[k]   = (query_addr >> k) & 1                 k in [0, 16)
    w[s]    = prod_k (1 - |x[b, s, 96+k] - qb[k]|)
    out[b,d]= sum_s w[s] * x[b, s, d]               d in [0, 96)

Sharding: data-parallel over batch, one NeuronCore per batch element
(B == 8 == n_cores), no collectives.

Per-core plan (x_core [32768, 128] fp32 in HBM):
  - slab layout: partition p holds the 256 consecutive rows
    s = p*256 + i.  Each DMA wave loads a block of rows [r0, r0+Wr) for
    all partitions; the per-partition source is one contiguous Wr*512 B
    run in HBM, so every wave is 128 large contiguous descriptors
    (up to 24 KiB each) instead of many 2 KiB ones.  Per-SDMA-engine
    rate goes from ~24 GB/s (2 KiB descs) to ~26 GB/s, near the ~27
    GB/s port limit (~410 GB/s aggregate when the sibling NeuronCore
    on the same HBM stack is not streaming concurrently).
  - all data waves ride ONE HWDGE ring (see WS_ROWS comment): strict
    FIFO makes waves complete in program order for the in-order DVE
    consumer, and the wave schedule tapers 64 -> 4 rows toward the
    tail so each wave's weight chain finishes before the next wave's
    completion lands; the post-stream drain is just the 4-row chain +
    one matmul + PSUM drain.  No head ramp: compute has ~2x slack
    against the stream, so even a 64-row first wave starts the
    pipeline early enough, and fewer waves mean less fixed cost.
  - match weights, per wave: d = bits - qb (DVE, fp16 out),
    na = min(-d, d) = -|d| (DVE stt), m = 1 + na (DVE), then ONE
    product-reduction over the 16 factors (DVE tensor_reduce mult)
    giving w[p, i] in f32r.  fp16 intermediates add ~1.5e-3 L2 error,
    well inside the 2e-2 gate, and cannot overflow (sum log2|m_k| is
    far below 16 for N(0,1) inputs).
  - einsum on TensorE: per 4-row group,
    psum[4, 384] += w[:, 4j:4j+4].T @ v[:, 4j:4j+4, 0:96]  (float32r,
    1 cycle/row at N=384), accumulated across all 64 groups in one
    PSUM bank.  Only the diagonal 96-blocks (r == r') are wanted; the
    host extracts and sums them (24 junk floats per row ignored).
  - mode "f32" is an exact-fp32 fallback (fp32 weight chain, fp32 PE).
"""

import os
import sys

if "/opt/trn_rl_repo" not in sys.path:
    sys.path.insert(0, "/opt/trn_rl_repo")

import numpy as np

S, D = 32768, 128
VD = 96          # value payload dims
NBITS = 16
BIT0 = 96
P = 128          # partitions
IPP = S // P     # 256 seq rows per partition (slab layout)
R = 4            # seq rows per matmul (diagonal trick)
C = R

# Wave sizes in rows-per-partition.  All data waves ride ONE HWDGE
# ring (Sync's qSPDynamicHW): the SDMA engines drain a ring strictly
# FIFO, so waves complete in program order and the in-order DVE/PE
# consumers never stall on a late out-of-order wave (two rings
# interleave per-descriptor, which starves whichever ring holds the
# smaller descriptors).  The tail tapers so each wave's DVE chain
# (~rows*0.07 + 0.35 us) finishes before the next wave's completion
# lands (next_rows * 0.158 us later), leaving only a 4-row chain +
# one matmul + PSUM drain after the last DMA byte.  No head ramp:
# compute has ~2x slack, and every extra wave costs ~0.35 us of DVE
# fixed time plus a DMA issue.
WS_ROWS = [64, 64, 48, 32, 24, 12, 8, 4]
assert sum(WS_ROWS) == IPP
assert all(w % R == 0 for w in WS_ROWS)

NCORES = 8

# "f32r" : float32r matmuls (1 cycle/row, ~5e-4 rel err)
# "f32"  : plain fp32 matmuls (4 cycles/row, exact)
MM_MODE = os.environ.get("BMA_MM_MODE", "f32r")

_CACHE = {}


def _build(mode):
    import concourse.bacc as bacc
    import concourse.mybir as mybir
    import concourse.tile as tile

    f32 = mybir.dt.float32
    x_dt = mybir.dt.float32r if mode == "f32r" else f32
    # fp16 for the match-weight chain: 2x DVE throughput, and the
    # rounding error (~1e-3 L2 on the output) is far inside the 2e-2
    # gate.  No overflow risk: sum(log2|m_k|) stays far below 16.
    w_dt = mybir.dt.float16 if mode == "f32r" else f32

    nc = bacc.Bacc("TRN2", target_bir_lowering=False, debug=False)
    x = nc.dram_tensor("x", [S, D], x_dt, kind="ExternalInput")
    cq = nc.dram_tensor("cq", [P, NBITS], f32, kind="ExternalInput")
    out = nc.dram_tensor("out", [C, C * VD], f32, kind="ExternalOutput")

    # [128(part), 256*128]; partition p's source is one contiguous
    # 128 KiB run in HBM, so any row-block slice is a single large
    # contiguous descriptor per partition.
    xr = x.ap().rearrange("(p i) d -> p (i d)", p=P)

    n_mm = IPP // R  # 64 accumulating matmuls
    WMAX = max(WS_ROWS)
    NVT = 5  # DMA-ahead depth (manual ring of persistent vt tiles)
    with tile.TileContext(nc) as tc:
        with (
            tc.tile_pool(name="const", bufs=1) as cpool,
            tc.tile_pool(name="v", bufs=1) as vpool,
            tc.tile_pool(name="wk", bufs=1) as wpool,
            tc.tile_pool(name="ps", bufs=1, space="PSUM") as ppool,
            tc.tile_pool(name="res", bufs=1) as rpool,
        ):
            cqt = cpool.tile([P, 1, NBITS], f32)
            # cq rides the ACT HWDGE ring so wave 0's descriptors on
            # the SP ring are generated without queueing behind it.
            nc.scalar.dma_start(cqt[:], cq.ap().rearrange("p (a k) -> p a k", a=1))

            acc = ppool.tile([C, C * VD], f32)

            # Persistent tiles, sliced per wave, instead of per-wave
            # pool.tile() calls: every logical tile costs a release +
            # one semaphore hop PER ENGINE in Tile's end-of-kernel
            # drain (measured ~130 ns each x ~57 tiles = ~7.5 us of
            # teardown).  The chain tiles (d/na/m) are produced and
            # consumed only by the in-order DVE stream, so single
            # buffers add no stalls; w is read cross-engine by PE so
            # it double-buffers; vt keeps an NVT-deep manual ring for
            # DMA-ahead, equivalent to the old pool bufs.
            dt_ = wpool.tile([P, WMAX, NBITS], w_dt, tag="d")
            nat = wpool.tile([P, WMAX, NBITS], w_dt, tag="na")
            mt = wpool.tile([P, WMAX, NBITS], w_dt, tag="m")
            wts = [
                wpool.tile([P, WMAX, 1], x_dt, tag=f"w{i}", name=f"w{i}")
                for i in range(2)
            ]
            vts = [
                vpool.tile([P, WMAX * D], x_dt, tag=f"vt{i}", name=f"vt{i}")
                for i in range(NVT)
            ]

            g = 0
            r0 = 0
            for ib, Wr in enumerate(WS_ROWS):
                vt = vts[ib % NVT][:, 0 : Wr * D]
                nc.sync.dma_start(vt, xr[:, r0 * D : (r0 + Wr) * D])
                vr = vt.rearrange("p (i d) -> p i d", d=D)

                bits = vr[:, :, BIT0 : BIT0 + NBITS]
                if mode == "f32r":
                    bits = bits.bitcast(f32)
                d = dt_[:, 0:Wr, :]
                nc.vector.tensor_sub(d, bits, cqt[:].broadcast_to([P, Wr, NBITS]))
                # na = min(-d, d) = -|d| on DVE; ACT is kept free so its
                # HWDGE descriptor pushes are never blocked behind compute
                na = nat[:, 0:Wr, :]
                nc.vector.scalar_tensor_tensor(
                    na, d, -1.0, d,
                    op0=mybir.AluOpType.mult, op1=mybir.AluOpType.min,
                )
                m = mt[:, 0:Wr, :]
                nc.vector.tensor_scalar(
                    m, na, 1.0, None, op0=mybir.AluOpType.add,
                )
                # single DVE product-reduce over the 16 match factors,
                # written in the matmul dtype so the verifier sees an
                # f32r producer
                w = wts[ib % 2][:, 0:Wr, :]
                nc.vector.tensor_reduce(
                    w, m, axis=mybir.AxisListType.X,
                    op=mybir.AluOpType.mult,
                )

                for j in range(Wr // R):
                    lhsT = w[:, j * R : (j + 1) * R, 0]     # [128, 4]
                    rhs = vr[:, j * R : (j + 1) * R, 0:VD]  # [128, 4, 96]
                    nc.tensor.matmul(
                        acc[:],
                        lhsT,
                        rhs,
                        start=(g == 0),
                        stop=(g == n_mm - 1),
                    )
                    g += 1
                r0 += Wr

            res = rpool.tile([C, C * VD], f32)
            nc.vector.tensor_copy(res[:], acc[:])
            nc.sync.dma_start(out.ap(), res[:])

    nc.compile()
    return nc


def _get_nc(mode):
    if mode not in _CACHE:
        _CACHE[mode] = _build(mode)
    return _CACHE[mode]


def run(x, query_addr, trace=False, mode=None):
    """Returns (output [B, 96] float32, BassKernelResults)."""
    from concourse.bass_utils import run_bass_kernel_spmd

    mode = mode or MM_MODE
    x = np.asarray(x)
    qa = int(np.asarray(query_addr))
    assert x.shape == (NCORES, S, D), x.shape

    qb = np.array([(qa >> k) & 1 for k in range(NBITS)], dtype=np.float32)
    cq = np.ascontiguousarray(np.broadcast_to(qb, (P, NBITS)))

    nc = _get_nc(mode)
    in_maps = [
        {"x": np.ascontiguousarray(x[b], dtype=np.float32), "cq": cq}
        for b in range(NCORES)
    ]
    if not trace:
        # A stray BASS_TRACE in the env would route run_bass_kernel_spmd
        # into the NTFF-hook path, which needs antenv.axon_hooks (absent
        # in this image unless test.py installs a shim).
        os.environ["BASS_NEVER_TRACE"] = "1"
    else:
        os.environ.pop("BASS_NEVER_TRACE", None)
    kres = run_bass_kernel_spmd(nc, in_maps, list(range(NCORES)), trace=trace)

    outs = []
    for r in kres.results:
        o = np.asarray(r["out"]).reshape(C, C, VD)
        outs.append(o[np.arange(C), np.arange(C)].sum(axis=0))
    return np.stack(outs).astype(np.float32), kres


def kernel(x, query_addr):
    return run(x, query_addr)[0]
